# revision 1
# baseline (speedup 1.0000x reference)
"""BigBirdEncoder Trainium2 kernel.

Data-parallel over batch: 8 NeuronCores, core i computes sample i end-to-end
(no collectives). Per core: embedding gather (indirect DMA), 2x
(pre-RMSNorm block-sparse attention + MLP), final RMSNorm.

Layout strategy (per core):
  - Residual xT kept D-major [256, 2048] fp32, SBUF-resident.
  - RMSNorm: sum-of-squares via ones-matmul (partition reduction on PE),
    rstd = exp(-0.5*ln(msq+eps)) on ACT, broadcast over partitions via K=1
    outer-product matmul (bf16 hi/lo split for precision).
  - QKV: qT,kT D-major bf16 (lhsT=weights); v token-major bf16 with a fused
    ones column per head (v_aug) so the o-matmul also produces softmax
    denominators (33rd output row).
  - Scores computed transposed (sT = [kv, q]) per (head, 2 query blocks):
    global blocks batched, window+random as even/odd-aligned block chunks.
    exp on ACT in [128, 2048] batches, no max-subtraction (|s| < 10).
  - o-matmuls: head pairs column-packed at partition offsets 0/64 (M=33);
    denominator rows at 32/96; normalization deferred: reciprocal on DVE,
    partition-broadcast via K=2 selector matmul, one in-place multiply.
  - wo contraction uses zero-padded weight rows so garbage psum rows
    (33..63, 97..127) contribute nothing.
"""

import os
import sys

for _p in ("/opt/trn_rl_repo", "/root/.axon_site/_ro/trn_rl_repo"):
    if os.path.isdir(_p) and _p not in sys.path:
        sys.path.insert(0, _p)

import numpy as np
import ml_dtypes

import concourse.bass as bass
import concourse.mybir as mybir
import concourse.tile as tile
from concourse import bacc, bass_utils

BF16 = mybir.dt.bfloat16
F32 = mybir.dt.float32
I32 = mybir.dt.int32
AF = mybir.ActivationFunctionType
ALU = mybir.AluOpType

S = 2048
D = 256
H = 8
DH = 32
BS = 64
NB = 32
FF = 1024
L = 2
EPS = 1e-8
NT = 16          # 128-token tiles
NCORES = 8
V = 32000


def _plan_attention(rand_idx):
    """Per query block n: dedup'd window+random kv-block chunks.

    Returns plans[n] = list of (b0, nb, mult, g, po):
      b0 first block id, nb blocks (1 or 2), mult = multiplicity,
      g = column group (0..2), po = partition offset in the chunk column (0/64).
    Invariants: nb==2 chunks sit at po=0; single chunks with b0==0 sit at po=0.
    """
    plans = []
    for n in range(NB):
        blocks = sorted([
            max(n - 1, 0), n, min(n + 1, NB - 1),
            int(rand_idx[n, 0]), int(rand_idx[n, 1]),
        ])
        # dedup with multiplicity
        uniq = []
        for b in blocks:
            if uniq and uniq[-1][0] == b:
                uniq[-1][1] += 1
            else:
                uniq.append([b, 1])
        # pair adjacent blocks with equal multiplicity (any parity; odd pairs
        # are served by the 64-shifted v copy)
        chunks = []
        i = 0
        while i < len(uniq):
            if (i + 1 < len(uniq) and uniq[i + 1][0] == uniq[i][0] + 1
                    and uniq[i][1] == uniq[i + 1][1]):
                chunks.append((uniq[i][0], 2, uniq[i][1]))
                i += 2
            else:
                chunks.append((uniq[i][0], 1, uniq[i][1]))
                i += 1
        # placement: pairs take a full group; singles pack two per group
        # (lower half po=0, upper half po=64). Constraints: a b0==0 single
        # has no shifted-v copy and must sit at po=0; a b0==NB-1 odd single
        # must sit at po=64.
        placement = []
        g = 0
        lowers, uppers = [], []
        pairs = [c for c in chunks if c[1] == 2]
        singles = [c for c in chunks if c[1] == 1]
        singles.sort(key=lambda c: 0 if c[0] == 0 else (1 if c[0] == NB - 1 else 2))
        for (b0, nb, m) in pairs:
            placement.append((b0, 2, m, g, 0))
            g += 1
        for (b0, nb, m) in singles:
            if b0 == 0:
                want = 0
            elif b0 == NB - 1 and b0 % 2 == 1:
                want = 64
            else:
                want = None
            if want == 0 or (want is None and not uppers and lowers):
                gg = lowers.pop(0) if lowers else None
                if gg is None:
                    gg = g
                    g += 1
                    uppers.append(gg)
                placement.append((b0, 1, m, gg, 0))
            else:
                gg = uppers.pop(0) if uppers else None
                if gg is None:
                    gg = g
                    g += 1
                    lowers.append(gg)
                    if want == 64:
                        placement.append((b0, 1, m, gg, 64))
                        lowers.pop()
                        lowers.append(gg)
                        continue
                placement.append((b0, 1, m, gg, 64))
        assert g <= 3, (n, chunks, placement)
        # sort the two singles of a group so po=0 block < po=64 block
        # (merged score matmul needs a positive-stride two-block AP).
        # Pinned placements (b0==0 at po=0, odd b0==NB-1 at po=64) are
        # already sorted by construction.
        by_g = {}
        for i, e in enumerate(placement):
            by_g.setdefault(e[3], []).append(i)
        for gg, idxs in by_g.items():
            sing = [i for i in idxs if placement[i][1] == 1]
            if len(sing) == 2:
                lo_i = sing[0] if placement[sing[0]][4] == 0 else sing[1]
                hi_i = sing[1] if lo_i == sing[0] else sing[0]
                if placement[lo_i][0] > placement[hi_i][0]:
                    bl, bh = placement[lo_i], placement[hi_i]
                    assert bl[0] != 0 and not (bh[0] == NB - 1 and bh[0] % 2)
                    placement[lo_i] = (bl[0], 1, bl[2], gg, 64)
                    placement[hi_i] = (bh[0], 1, bh[2], gg, 0)
        # merged score-matmul spec per column group: ('pair', b0) covers
        # rows 0..127 with blocks (b0, b0+1); ('two', lo, hi) covers rows
        # 0..63 with block lo and 64..127 with block hi (hi > lo).
        sg = []
        for gg in range(3):
            ent = [placement[i] for i in by_g.get(gg, [])]
            pair = [e for e in ent if e[1] == 2]
            if pair:
                sg.append(("pair", pair[0][0]))
            else:
                lo = next((e[0] for e in ent if e[4] == 0), None)
                hi = next((e[0] for e in ent if e[4] == 64), None)
                sg.append(("two", lo, hi))
        plans.append((placement, sg))
    return plans


def _patch_act_tables():
    """Make Ln and Exp resolve to the shared natural_log_exp_and_others
    table set (greedy first-containing-set selection otherwise thrashes
    between natural_log and exp_and_others on every RMSNorm: ~2.7us per
    reload). Only the func membership used for set *selection* is edited;
    list order — and therefore every act_func_set_id — is preserved, and
    the chosen set genuinely contains both functions."""
    from concourse import hw_specs
    orig = hw_specs.get_activation_tables

    def patched(arch):
        t = orig(arch)
        if "natural_log_exp_and_others" in t:
            t.get("exp_and_others", set()).discard(AF.Exp)
            t.get("natural_log", set()).discard(AF.Ln)
        return t

    hw_specs.get_activation_tables = patched
    bacc.get_activation_tables = patched
    return orig


def build_kernel(rand_idx, reps=1):
    phase = os.environ.get("K_PHASE", "full")
    plans = _plan_attention(rand_idx)
    nc = bacc.Bacc("TRN2", target_bir_lowering=False, debug=False,
                   num_devices=NCORES, detect_race_conditions=False)

    ids_d = nc.dram_tensor("ids", [128, NT], I32, kind="ExternalInput").ap()
    emb_d = nc.dram_tensor("emb", [V, D], F32, kind="ExternalInput").ap()
    wq_d = nc.dram_tensor("wq", [L, 2, 128, D], BF16, kind="ExternalInput").ap()
    wk_d = nc.dram_tensor("wk", [L, 2, 128, D], BF16, kind="ExternalInput").ap()
    wv_d = nc.dram_tensor("wv", [L, 2, 128, D], BF16, kind="ExternalInput").ap()
    wop_d = nc.dram_tensor("wop", [L, 4, 128, D], BF16, kind="ExternalInput").ap()
    w1_d = nc.dram_tensor("w1", [L, 2, 128, FF], BF16, kind="ExternalInput").ap()
    w2_d = nc.dram_tensor("w2", [L, 8, 128, D], BF16, kind="ExternalInput").ap()
    onesr_d = nc.dram_tensor("onesr", [1, 128], BF16, kind="ExternalInput").ap()
    onesc_d = nc.dram_tensor("onesc", [128, 1], BF16, kind="ExternalInput").ap()
    ident_d = nc.dram_tensor("ident", [128, 128], F32, kind="ExternalInput").ap()
    sel2_d = nc.dram_tensor("sel2", [2, 128], BF16, kind="ExternalInput").ap()
    fln_d = nc.dram_tensor("fln", [128, 2], F32, kind="ExternalInput").ap()
    out_d = nc.dram_tensor("out", [S, D], F32, kind="ExternalOutput").ap()

    from contextlib import ExitStack
    with tile.TileContext(nc) as tc, ExitStack() as ctx:
        ep = ctx.enter_context
        consts = ep(tc.tile_pool(name="consts", bufs=1))
        wpool = ep(tc.tile_pool(name="wpool", bufs=1))
        xpool = ep(tc.tile_pool(name="xpool", bufs=1))
        expp = ep(tc.tile_pool(name="expp", bufs=4))
        rp = ep(tc.tile_pool(name="rp", bufs=1))
        stage = ep(tc.tile_pool(name="stage", bufs=2))
        small = ep(tc.tile_pool(name="small", bufs=1))
        psum = ep(tc.tile_pool(name="psum", bufs=1, space="PSUM"))
        if True:
            # ---- constants ----
            ids_t = consts.tile([128, NT], I32)
            nc.sync.dma_start(out=ids_t, in_=ids_d)
            onesr = consts.tile([1, 128], BF16)
            nc.sync.dma_start(out=onesr, in_=onesr_d)
            onesc = consts.tile([128, 1], BF16)
            nc.sync.dma_start(out=onesc, in_=onesc_d)
            ident = consts.tile([128, 128], F32)
            nc.sync.dma_start(out=ident, in_=ident_d)
            sel2t = consts.tile([2, 128], BF16)
            nc.sync.dma_start(out=sel2t, in_=sel2_d)
            fln_t = consts.tile([128, 2], F32)
            nc.sync.dma_start(out=fln_t, in_=fln_d)
            eps_t = consts.tile([1, 1], F32)
            nc.vector.memset(eps_t, EPS)

            # ---- persistent activations ----
            xT = xpool.tile([128, 2, S], F32, tag="xT")
            qTt = xpool.tile([128, 2, S], BF16, tag="qT")
            kTt = xpool.tile([128, 2, S], BF16, tag="kT")
            oT = xpool.tile([128, 4, S], BF16, tag="oT")
            nc.vector.memset(oT, 0.0)
            VW = H * 33 + 32          # 296: 8x(32 v + 1 ones) + zero tail
            vA = xpool.tile([128, NT, VW], BF16, tag="vA")
            vS = xpool.tile([128, NT - 1, VW], BF16, tag="vS")
            # ones columns + zero tail persist across layers (evacs only
            # write the 32 v columns of each head)
            vA4 = vA[:, :, 0:H * 33].rearrange("p a (h c) -> p a h c", c=33)
            nc.vector.memset(vA4[:, :, :, 32:33], 1.0)
            nc.vector.memset(vA[:, :, H * 33:VW], 0.0)

            for _rep in range(reps):
                # ---- embedding gather + transpose to D-major ----
                for t in range(NT):
                    xtok = stage.tile([128, D], F32, tag="xtok",
                                      bufs=4)
                    nc.gpsimd.indirect_dma_start(
                        out=xtok, out_offset=None, in_=emb_d,
                        in_offset=bass.IndirectOffsetOnAxis(ap=ids_t[:, t:t + 1], axis=0),
                    )
                    for c in range(2):
                        tp = psum.tile([128, 512], F32, tag="b1", bufs=2)
                        nc.tensor.transpose(tp[:, 0:128],
                                            xtok[:, c * 128:(c + 1) * 128], ident)
                        nc.vector.tensor_copy(xT[:, c, t * 128:(t + 1) * 128],
                                              tp[:, 0:128])

                def rmsnorm_factors():
                    """-> (rh, rl) bf16 hi/lo split of per-token rstd [1, S]."""
                    sq = xpool.tile([128, 2, S], BF16, tag="xn")
                    for po in range(2):
                        nc.vector.tensor_tensor(sq[:, po, :], xT[:, po, :],
                                                xT[:, po, :], op=ALU.mult)
                    rstd = small.tile([1, S], F32, tag="rstd")
                    for nt4 in range(4):
                        sl = slice(nt4 * 512, (nt4 + 1) * 512)
                        ssq = psum.tile([1, 512], F32, tag="b1", bufs=2)
                        for c in range(2):
                            nc.tensor.matmul(ssq, onesc, sq[:, c, sl],
                                             start=(c == 0), stop=(c == 1))
                        nc.scalar.activation(rstd[:, sl], ssq, AF.Ln,
                                             bias=eps_t[:, :], scale=1.0 / D)
                    nc.scalar.activation(rstd, rstd, AF.Exp, scale=-0.5)
                    rh = small.tile([1, S], BF16, tag="rh")
                    nc.vector.tensor_copy(rh, rstd)
                    rl = small.tile([1, S], BF16, tag="rl")
                    nc.vector.tensor_tensor(rl, rstd, rh, op=ALU.subtract)
                    return rh, rl

                def norm_slice(sq_t, xn_t, rstd, rh, rl, nt4):
                    """Per-512-slice RMSNorm: fills xn_t[:, :, sl] from
                    xT[:, :, sl]; pipelines with surrounding matmul work."""
                    sl = slice(nt4 * 512, (nt4 + 1) * 512)
                    for po in range(2):
                        nc.vector.tensor_tensor(sq_t[:, po, sl],
                                                xT[:, po, sl], xT[:, po, sl],
                                                op=ALU.mult)
                    ssq = psum.tile([1, 512], F32, tag="b1", bufs=2)
                    for c in range(2):
                        nc.tensor.matmul(ssq, onesc, sq_t[:, c, sl],
                                         start=(c == 0), stop=(c == 1))
                    nc.scalar.activation(rstd[:, sl], ssq, AF.Ln,
                                         bias=eps_t[:, :], scale=1.0 / D)
                    nc.scalar.activation(rstd[:, sl], rstd[:, sl],
                                         AF.Exp, scale=-0.5)
                    nc.vector.tensor_copy(rh[:, sl], rstd[:, sl])
                    nc.vector.tensor_tensor(rl[:, sl], rstd[:, sl], rh[:, sl],
                                            op=ALU.subtract)
                    bc = psum.tile([128, 512], F32, tag="b1", bufs=2)
                    nc.tensor.matmul(bc, onesr, rh[:, sl],
                                     start=True, stop=False)
                    nc.tensor.matmul(bc, onesr, rl[:, sl],
                                     start=False, stop=True)
                    for po in range(2):
                        nc.vector.tensor_tensor(xn_t[:, po, sl],
                                                xT[:, po, sl], bc,
                                                op=ALU.mult)

                def make_xn():
                    rh, rl = rmsnorm_factors()
                    xn = xpool.tile([128, 2, S], BF16, tag="xn")
                    for nt4 in range(4):
                        sl = slice(nt4 * 512, (nt4 + 1) * 512)
                        bc = psum.tile([128, 512], F32, tag="b1", bufs=2)
                        nc.tensor.matmul(bc, onesr, rh[:, sl],
                                         start=True, stop=False)
                        nc.tensor.matmul(bc, onesr, rl[:, sl],
                                         start=False, stop=True)
                        for po in range(2):
                            nc.vector.tensor_tensor(xn[:, po, sl],
                                                    xT[:, po, sl], bc,
                                                    op=ALU.mult)
                    return xn

                n_layers = 0 if phase == "embed" else (1 if phase != "full" else L)
                for l in range(n_layers):
                    # ---- layer weights ----
                    wq_t = wpool.tile([128, 2, D], BF16, tag="wq")
                    wk_t = wpool.tile([128, 2, D], BF16, tag="wk")
                    wv_t = wpool.tile([128, 2, D], BF16, tag="wv")
                    for c in range(2):
                        nc.sync.dma_start(out=wq_t[:, c, :], in_=wq_d[l, c])
                        nc.sync.dma_start(out=wk_t[:, c, :], in_=wk_d[l, c])
                        nc.sync.dma_start(out=wv_t[:, c, :], in_=wv_d[l, c])
                    wop_t = wpool.tile([128, 4, D], BF16, tag="wop")
                    for hp in range(4):
                        nc.sync.dma_start(out=wop_t[:, hp, :], in_=wop_d[l, hp])
                    w1_t = wpool.tile([128, 2, FF], BF16, tag="w1")
                    for c in range(2):
                        nc.sync.dma_start(out=w1_t[:, c, :], in_=w1_d[l, c])
                    w2_t = wpool.tile([128, 8, D], BF16, tag="w2")
                    for kc in range(8):
                        nc.sync.dma_start(out=w2_t[:, kc, :], in_=w2_d[l, kc])

                    # ---- norm 1 + QKV, fused per 512-token slice ----
                    sq_t = xpool.tile([128, 2, S], BF16, tag="xn")
                    xn = xpool.tile([128, 2, S], BF16, tag="xnb")
                    rstd = small.tile([1, S], F32, tag="rstd")
                    rh = small.tile([1, S], BF16, tag="rh")
                    rl = small.tile([1, S], BF16, tag="rl")
                    norm_slice(sq_t, xn, rstd, rh, rl, 0)
                    for nt in range(4):
                        if nt + 1 < 4:
                            norm_slice(sq_t, xn, rstd, rh, rl, nt + 1)
                        sl = slice(nt * 512, (nt + 1) * 512)
                        for (wt, dstT) in ((wq_t, qTt), (wk_t, kTt)):
                            for po in range(2):
                                pp = psum.tile([128, 512], F32, tag="b1",
                                               bufs=2)
                                for c in range(2):
                                    nc.tensor.matmul(
                                        pp,
                                        wt[:, c, po * 128:(po + 1) * 128],
                                        xn[:, c, sl],
                                        start=(c == 0), stop=(c == 1))
                                nc.vector.tensor_copy(dstT[:, po, sl], pp)
                        for sg in (2 * nt, 2 * nt + 1):
                            vp = psum.tile([128, 512], F32, tag="b1", bufs=2)
                            for stl in range(2):
                                st = sg * 2 + stl
                                for c in range(2):
                                    nc.tensor.matmul(
                                        vp[:, stl * 256:(stl + 1) * 256],
                                        xn[:, c, st * 128:(st + 1) * 128],
                                        wv_t[:, c, :],
                                        start=(c == 0), stop=(c == 1))
                            nc.vector.tensor_copy(
                                vA[:, sg * 2:(sg + 1) * 2, 0:H * 33]
                                .rearrange("p a (h c) -> p a h c", c=33)[:, :, :, 0:32],
                                vp.rearrange("p (a h c) -> p a h c", a=2, c=32))
                    # shifted-by-64 copy of v_aug (serves odd-aligned chunks)
                    nc.gpsimd.dma_start(out=vS[0:64, :, :],
                                        in_=vA[64:128, 0:NT - 1, :])
                    nc.gpsimd.dma_start(out=vS[64:128, :, :],
                                        in_=vA[0:64, 1:NT, :])

                    def v_slice(b0, nb, po, h):
                        """lhsT [64*nb, 64] for kv tokens [64*b0, 64*(b0+nb))
                        readable at partition offset po. 64-wide so the o-matmul
                        writes full partition halves (cols 33+ hit other heads'
                        data / the zero tail; those rows are killed by wo_pad)."""
                        hs = slice(h * 33, h * 33 + 64)
                        if nb == 2:
                            if b0 % 2 == 0:
                                return vA[:, b0 // 2, hs]
                            return vS[:, (b0 - 1) // 2, hs]
                        if po == 64 * (b0 % 2):
                            return vA[po:po + 64, b0 // 2, hs]
                        if b0 % 2 == 1:      # odd block at po=0 via shifted copy
                            return vS[0:64, (b0 - 1) // 2, hs]
                        # even block at po=64 via shifted copy (b0 >= 2 guaranteed)
                        return vS[64:128, b0 // 2 - 1, hs]

                    if phase == "qkv":
                        break
                    sub = phase[4:] if phase.startswith("attn") else ""
                    # ---- block-sparse attention ----
                    # Pipelined at (qt, hg)-group granularity: the PE stream
                    # is [scores(i), AV(i-1), (den/norm/wo when a qt
                    # completes)], so exp(i) on ACT overlaps AV(i-1)/norm/wo
                    # on PE and the PE never waits on the scalar engine.
                    def emit_scores(qt, hg):
                        ets = {}
                        for g2 in range(4):
                            j2 = qt * 4 + g2
                            for ph in range(2):
                                sc = psum.tile([128, 2, 512], F32, tag="sc2",
                                               bufs=2)
                                hlmms = {0: [], 1: []}
                                for hl in range(2):
                                    hh = ph * 2 + hl
                                    pb = 32 * hh
                                    hlmms[hl].append((
                                        sc[0:128, hl, 0:128],
                                        kTt[pb:pb + 32, hg, 0:128],
                                        qTt[pb:pb + 32, hg,
                                            j2 * 128:(j2 + 1) * 128],
                                        (pb, 0)))
                                    for ln_ in range(2):
                                        n = 2 * j2 + ln_
                                        qn = qTt[pb:pb + 32, hg,
                                                 n * 64:(n + 1) * 64]
                                        for g in range(3):
                                            spec = plans[n][1][g]
                                            co = 128 + ln_ * 192 + g * 64
                                            if spec[0] == "pair":
                                                b0 = spec[1]
                                                hlmms[hl].append((
                                                    sc[0:128, hl, co:co + 64],
                                                    kTt[pb:pb + 32, hg,
                                                        b0 * 64:(b0 + 2) * 64],
                                                    qn, (pb, 0)))
                                            else:
                                                # unused (fill) halves are
                                                # skipped: exp reads stale
                                                # psum there, and no AV
                                                # matmul ever consumes
                                                # those et slots
                                                for po, b0 in ((0, spec[1]),
                                                               (64, spec[2])):
                                                    if b0 is None:
                                                        continue
                                                    hlmms[hl].append((
                                                        sc[po:po + 64, hl,
                                                           co:co + 64],
                                                        kTt[pb:pb + 32, hg,
                                                            b0 * 64:
                                                            (b0 + 1) * 64],
                                                        qn, (pb, po)))
                                # interleave the two hl chains (distinct
                                # 32-row K-strips -> overlap in the array)
                                for i in range(max(len(hlmms[0]),
                                                   len(hlmms[1]))):
                                    for hl in range(2):
                                        if i < len(hlmms[hl]):
                                            o_ap, l_ap, r_ap, tpos = hlmms[hl][i]
                                            nc.tensor.matmul(
                                                o_ap, l_ap, r_ap,
                                                tile_position=tpos,
                                                start=True, stop=True)
                                et = expp.tile([128, 2, 512], BF16,
                                               tag="expt", bufs=16)
                                nc.scalar.activation(et, sc, AF.Exp)
                                ets[(g2, ph)] = et
                        return ets

                    def emit_av(qt, hg, ets, dqs):
                        ops = {}
                        chains = {}   # (ph, hl) -> ordered matmul list
                        for ph in range(2):
                            op_ = psum.tile([128, 512], F32, tag="avp", bufs=2)
                            ops[ph] = op_
                            mms = []   # (out, lhsT, rhs, tpos, region=hl)
                            for hl in range(2):
                                for g2 in range(4):
                                    h = hg * 4 + ph * 2 + hl
                                    mms.append(((
                                        op_[hl * 64:hl * 64 + 64,
                                            g2 * 128:(g2 + 1) * 128],
                                        vA[:, 0, h * 33:h * 33 + 64],
                                        ets[(g2, ph)][:, hl, 0:128],
                                        (0, hl * 64)), hl))
                            for hl in range(2):
                                # row-0 chunks first, then a full-height
                                # zero-spacer, then row-64 singles:
                                # row-disjoint K=64 matmuls touching the same
                                # psum region must not overlap in the PE.
                                row0, row64 = [], []
                                for g2 in range(4):
                                    for ln_ in range(2):
                                        n = 2 * (qt * 4 + g2) + ln_
                                        for (b0, nb, m, g, po) in plans[n][0]:
                                            co = 128 + ln_ * 192 + g * 64
                                            h = hg * 4 + ph * 2 + hl
                                            vsl = v_slice(b0, nb, po, h)
                                            ent = ((
                                                op_[hl * 64:hl * 64 + 64,
                                                    (g2 * 2 + ln_) * 64:
                                                    (g2 * 2 + ln_ + 1) * 64],
                                                vsl,
                                                ets[(g2, ph)][po:po + 64 * nb,
                                                              hl, co:co + 64],
                                                (po, hl * 64)), hl)
                                            dst = (row64
                                                   if (nb == 1 and po == 64)
                                                   else row0)
                                            for _ in range(m):
                                                dst.append(ent)
                                mms.extend(row0)
                                if row64:
                                    mms.append(((
                                        op_[hl * 64:hl * 64 + 32, 0:64],
                                        vA[:, 0, H * 33:H * 33 + 32],
                                        ets[(0, ph)][:, hl, 0:64],
                                        (0, hl * 64)), hl))
                                    mms.extend(row64)
                            chains[(ph, 0)] = [e for e in mms if e[1] == 0]
                            chains[(ph, 1)] = [e for e in mms if e[1] == 1]
                        # round-robin interleave the four (ph, hl) chains:
                        # consecutive instructions hit different psum banks /
                        # PE column strips so they can overlap in the array
                        # (order within each chain preserved)
                        order = [(0, 0), (0, 1), (1, 0), (1, 1)]
                        seq = []
                        i = 0
                        while any(i < len(chains[k]) for k in order):
                            for ph, hl in order:
                                if i < len(chains[(ph, hl)]):
                                    seq.append((ph, chains[(ph, hl)][i][0]))
                            i += 1
                        last_of = {}
                        seen = set()
                        for i, (ph, (_, _, _, tpos)) in enumerate(seq):
                            last_of[(ph, tpos[1])] = i
                        for i, (ph, (o_ap, l_ap, r_ap, tpos)) in enumerate(seq):
                            reg = (ph, tpos[1])
                            st = reg not in seen
                            seen.add(reg)
                            nc.tensor.matmul(
                                o_ap, l_ap, r_ap, tile_position=tpos,
                                start=st, stop=(last_of[reg] == i))
                        qsl = slice(qt * 512, (qt + 1) * 512)
                        for ph in range(2):
                            hp = hg * 2 + ph
                            nc.vector.tensor_copy(oT[:, hp, qsl], ops[ph])
                            if sub not in ("1", "2"):
                                # pack this hp's two denominator rows (bf16)
                                # to partitions 0/1 for the reciprocal
                                dqp = rp.tile([2, 512], BF16, tag="dq",
                                              bufs=8)
                                for r in range(2):
                                    nc.sync.dma_start(
                                        out=dqp[r:r + 1, :],
                                        in_=oT[32 + 64 * r:33 + 64 * r,
                                               hp, qsl])
                                dqs[hp] = dqp

                    def emit_norm_wo(qt, dqs):
                        if sub in ("1", "2"):
                            return
                        qsl = slice(qt * 512, (qt + 1) * 512)
                        dqRs = {}
                        for hp in range(4):
                            dqR = rp.tile([2, 512], BF16, tag="dqR", bufs=8)
                            with nc.allow_low_precision("softmax recip bf16"):
                                nc.vector.reciprocal(dqR, dqs[hp])
                            dqRs[hp] = dqR
                        if sub == "3":
                            return
                        for hp in range(4):
                            bc2 = psum.tile([128, 512], F32, tag="b1", bufs=2)
                            nc.tensor.matmul(bc2, sel2t, dqRs[hp],
                                             start=True, stop=True)
                            nc.vector.tensor_tensor(oT[:, hp, qsl],
                                                    oT[:, hp, qsl],
                                                    bc2, op=ALU.mult)
                        if sub == "4":
                            return
                        for po in range(2):
                            wp = psum.tile([128, 512], F32, tag="b1", bufs=2)
                            for hp_ in range(4):
                                nc.tensor.matmul(
                                    wp, wop_t[:, hp_, po * 128:(po + 1) * 128],
                                    oT[:, hp_, qsl],
                                    start=(hp_ == 0), stop=(hp_ == 3))
                            nc.vector.tensor_tensor(xT[:, po, qsl],
                                                    xT[:, po, qsl],
                                                    wp, op=ALU.add)

                    pend_av = None      # (qt, hg, ets, dqs)
                    pend_nrm = None     # (qt, dqs)
                    dqs_cur = None
                    for qt in range(4):
                        for hg in range(2):
                            if hg == 0:
                                dqs_cur = {}
                            ets = emit_scores(qt, hg)
                            if pend_nrm is not None:
                                emit_norm_wo(*pend_nrm)
                                pend_nrm = None
                            if pend_av is not None and sub != "1":
                                emit_av(*pend_av)
                                if pend_av[1] == 1:
                                    pend_nrm = (pend_av[0], pend_av[3])
                            pend_av = (qt, hg, ets, dqs_cur)
                    if pend_av is not None and sub != "1":
                        if pend_nrm is not None:
                            emit_norm_wo(*pend_nrm)
                            pend_nrm = None
                        emit_av(*pend_av)
                        emit_norm_wo(pend_av[0], pend_av[3])

                    if phase.startswith("attn"):
                        break
                    # ---- norm 2 + FFN, fused per 512-token slice ----
                    sq2 = xpool.tile([128, 2, S], BF16, tag="xn")
                    xn2 = xpool.tile([128, 2, S], BF16, tag="xnb")
                    rstd2 = small.tile([1, S], F32, tag="rstd")
                    rh2 = small.tile([1, S], BF16, tag="rh")
                    rl2 = small.tile([1, S], BF16, tag="rl")
                    norm_slice(sq2, xn2, rstd2, rh2, rl2, 0)
                    for nt in range(4):
                        if nt + 1 < 4:
                            norm_slice(sq2, xn2, rstd2, rh2, rl2, nt + 1)
                        sl = slice(nt * 512, (nt + 1) * 512)
                        fg = stage.tile([128, 8, 512], BF16, tag="f1g")
                        for po8 in range(8):
                            fp_ = psum.tile([128, 512], F32, tag="b1", bufs=2)
                            for c in range(2):
                                nc.tensor.matmul(
                                    fp_, w1_t[:, c, po8 * 128:(po8 + 1) * 128],
                                    xn2[:, c, sl],
                                    start=(c == 0), stop=(c == 1))
                            nc.scalar.activation(fg[:, po8, :], fp_,
                                                 AF.Gelu_apprx_tanh)
                        for po in range(2):
                            f2p = psum.tile([128, 512], F32, tag="b1", bufs=2)
                            for kc in range(8):
                                nc.tensor.matmul(
                                    f2p, w2_t[:, kc, po * 128:(po + 1) * 128],
                                    fg[:, kc, :],
                                    start=(kc == 0), stop=(kc == 7))
                            nc.vector.tensor_tensor(xT[:, po, sl], xT[:, po, sl],
                                                    f2p, op=ALU.add)

                # ---- final RMSNorm (with final_ln_w) + transpose out ----
                xnF = xpool.tile([128, 2, S], F32, tag="xnF")
                if phase == "full":
                    rh, rl = rmsnorm_factors()
                    for nt4 in range(4):
                        sl = slice(nt4 * 512, (nt4 + 1) * 512)
                        bcf = psum.tile([128, 512], F32, tag="b1", bufs=2)
                        nc.tensor.matmul(bcf, onesr, rh[:, sl],
                                         start=True, stop=False)
                        nc.tensor.matmul(bcf, onesr, rl[:, sl],
                                         start=False, stop=True)
                        for po in range(2):
                            nc.vector.tensor_tensor(xnF[:, po, sl],
                                                    xT[:, po, sl], bcf,
                                                    op=ALU.mult)
                            nc.vector.tensor_scalar(
                                out=xnF[:, po, sl], in0=xnF[:, po, sl],
                                scalar1=fln_t[:, po:po + 1], scalar2=None,
                                op0=ALU.mult)
                else:
                    for po in range(2):
                        nc.vector.tensor_copy(xnF[:, po, :], xT[:, po, :])
                for t in range(NT):
                    osb = stage.tile([128, D], F32, tag="osb")
                    for po in range(2):
                        tp = psum.tile([128, 512], F32, tag="b1", bufs=2)
                        nc.tensor.transpose(
                            tp[:, 0:128], xnF[:, po, t * 128:(t + 1) * 128], ident)
                        nc.vector.tensor_copy(osb[:, po * 128:(po + 1) * 128],
                                              tp[:, 0:128])
                    nc.sync.dma_start(out=out_d[t * 128:(t + 1) * 128, :], in_=osb)

    from concourse import hw_specs
    _orig = _patch_act_tables()
    try:
        nc.compile()
    finally:
        hw_specs.get_activation_tables = _orig
        bacc.get_activation_tables = _orig
    return nc


def prep_in_maps(inputs):
    bf = ml_dtypes.bfloat16
    ids = np.asarray(inputs["input_ids"]).astype(np.int32)          # [8, S]
    rand_idx = np.asarray(inputs["rand_idx"]).astype(np.int32)      # [NB, 2]
    emb = np.ascontiguousarray(np.asarray(inputs["emb"], np.float32))
    ln1 = np.asarray(inputs["ln1_w"], np.float32)
    ln2 = np.asarray(inputs["ln2_w"], np.float32)
    wq = np.asarray(inputs["wq"], np.float32)
    wk = np.asarray(inputs["wk"], np.float32)
    wv = np.asarray(inputs["wv"], np.float32)
    wo = np.asarray(inputs["wo"], np.float32)
    w1 = np.asarray(inputs["w1"], np.float32)
    w2 = np.asarray(inputs["w2"], np.float32)
    fln = np.asarray(inputs["final_ln_w"], np.float32)

    scale = 1.0 / np.sqrt(DH)
    wq_p = np.ascontiguousarray(
        (wq * ln1[:, :, None] * scale).reshape(L, 2, 128, D)).astype(bf)
    wk_p = np.ascontiguousarray(
        (wk * ln1[:, :, None]).reshape(L, 2, 128, D)).astype(bf)
    wv_p = np.ascontiguousarray(
        (wv * ln1[:, :, None]).reshape(L, 2, 128, D)).astype(bf)
    wop = np.zeros((L, 4, 128, D), np.float32)
    for hp in range(4):
        wop[:, hp, 0:32, :] = wo[:, 64 * hp:64 * hp + 32, :]
        wop[:, hp, 64:96, :] = wo[:, 64 * hp + 32:64 * hp + 64, :]
    wop = wop.astype(bf)
    w1_p = np.ascontiguousarray(
        (w1 * ln2[:, :, None]).reshape(L, 2, 128, FF)).astype(bf)
    w2_p = np.ascontiguousarray(w2.reshape(L, 8, 128, D)).astype(bf)

    sel2 = np.zeros((2, 128), bf)
    sel2[0, :64] = 1.0
    sel2[1, 64:] = 1.0
    common = {
        "emb": emb,
        "wq": wq_p, "wk": wk_p, "wv": wv_p, "wop": wop,
        "w1": w1_p, "w2": w2_p,
        "onesr": np.ones((1, 128), bf),
        "onesc": np.ones((128, 1), bf),
        "ident": np.eye(128, dtype=np.float32),
        "sel2": sel2,
        "fln": np.ascontiguousarray(fln.reshape(2, 128).T),
    }
    in_maps = []
    for c in range(NCORES):
        m = dict(common)
        m["ids"] = np.ascontiguousarray(ids[c].reshape(NT, 128).T)
        in_maps.append(m)
    return in_maps, rand_idx


_NC_CACHE = {}


def get_nc(rand_idx, reps=1):
    key = (os.environ.get("K_PHASE", "full"), os.environ.get("K_SC", "gwf"),
           os.environ.get("K_O", "gw"), os.environ.get("K_RECIP", ""),
           reps, rand_idx.tobytes())
    if key not in _NC_CACHE:
        _NC_CACHE[key] = build_kernel(rand_idx, reps=reps)
    return _NC_CACHE[key]


def kernel(**inputs):
    in_maps, rand_idx = prep_in_maps(inputs)
    nc = get_nc(rand_idx)
    res = bass_utils.run_bass_kernel_spmd(nc, in_maps, list(range(NCORES)),
                                          trace=False)
    out = np.stack([np.asarray(res.results[c]["out"], np.float32)
                    for c in range(NCORES)])
    return out

